# revision 13
# baseline (speedup 1.0000x reference)
"""Trainium2 Bass kernel for windowed (sink/ring-buffer) self-attention with RoPE.

Contract: kernel(**inputs) takes FULL unsharded inputs (as produced by the
problem's setup_inputs) and returns the FULL output [B, L, n, d].

Sharding: 12 heads x 1440 queries are split across 8 NeuronCores as
1.5 "head-units" per core: each core owns one full head (1440 queries) plus
half of a head shared with its pair core (720 queries). All cores run the
same SPMD program on differently-sliced inputs.

Device program (per core):
  - RoPE applied on-chip to q and the new k block (4 tensor ops per block,
    using host-precomputed cos/sin tables in a de-interleaved d-layout that
    turns the rotation into plain elementwise ops; the d-permutation cancels
    inside the QK^T contraction).
  - S^T = ka^T q computed in [kv, q] orientation (fp32r matmuls), exp on
    ScalarE straight out of PSUM, then OT = va^T P and softmax denominators
    accumulated in PSUM; final transpose back to [q, d] on TensorE with a
    per-partition reciprocal scale.
"""

import math

import numpy as np

P = 128
THETA = 10000.0
LOCAL_ATTN_SIZE = 15
SINK_SIZE = 1

QBLK = 768          # q columns per pass (2 psum banks: 512 + 256 chunks)
CHUNKS = ((0, 512), (512, 256))

_BUILD_CACHE = {}


# ----------------------------------------------------------------------------
# host-side planning (mirrors the reference's python-int index logic)
# ----------------------------------------------------------------------------

def _plan_cache_segments(current_start, global_end_index, local_end_index,
                         num_new, cache_len, frame_seqlen):
    """Return (segments, local_end, kv_start): list of (lo, hi) slices of the
    ORIGINAL cache arrays that make up the pre-new-token part of the attention
    window, mirroring reference.py's roll/evict logic."""
    current_end = current_start + num_new
    sink_tokens = SINK_SIZE * frame_seqlen
    max_attn = LOCAL_ATTN_SIZE * frame_seqlen
    if current_end > global_end_index and num_new + local_end_index > cache_len:
        n_evict = num_new + local_end_index - cache_len
        n_roll = local_end_index - n_evict - sink_tokens
        local_end = local_end_index + current_end - global_end_index - n_evict
        roll_lo, roll_hi = sink_tokens, sink_tokens + n_roll

        def old_index(i):
            return i + n_evict if roll_lo <= i < roll_hi else i
    else:
        local_end = local_end_index + current_end - global_end_index
        n_evict = 0

        def old_index(i):
            return i

    local_start = local_end - num_new
    kv_start = max(0, local_end - max_attn)
    # contiguous segments of old_index over [kv_start, local_start)
    segs = []
    i = kv_start
    while i < local_start:
        lo = old_index(i)
        j = i
        while j + 1 < local_start and old_index(j + 1) == old_index(j) + 1:
            j += 1
        segs.append((lo, lo + (j - i + 1)))
        i = j + 1
    return segs, local_end, kv_start


def _rope_cos_sin(L, d, grid_h, grid_w, start_frame):
    """cos/sin angle tables [L, d//2] matching reference make_freqs/rope_apply."""
    c = d // 2
    d1 = d - 4 * (d // 6)
    d2 = 2 * (d // 6)
    inv1 = THETA ** (-(np.arange(0, d1, 2, dtype=np.float32) / np.float32(d1)))
    inv2 = THETA ** (-(np.arange(0, d2, 2, dtype=np.float32) / np.float32(d2)))
    inv3 = inv2
    hw = grid_h * grid_w
    pos = np.arange(L)
    f = pos // hw + start_frame
    hh = (pos % hw) // grid_w
    ww = pos % grid_w
    ang = np.concatenate([
        f[:, None].astype(np.float32) * inv1[None, :],
        hh[:, None].astype(np.float32) * inv2[None, :],
        ww[:, None].astype(np.float32) * inv3[None, :],
    ], axis=1)
    assert ang.shape == (L, c)
    return np.cos(ang).astype(np.float32), np.sin(ang).astype(np.float32)


# ----------------------------------------------------------------------------
# device program
# ----------------------------------------------------------------------------

def _build_program(L, d, n_cache, n_kv):
    """Build the SPMD Bass program for one core.

    L: new-token count (1440); d: head dim (128); n_cache: cache rows in the
    window (9360); n_kv: total kv rows (10800)."""
    import concourse.bass as bass
    import concourse.mybir as mybir
    import concourse.tile as tile
    from concourse import bacc

    f32 = mybir.dt.float32
    f32r = mybir.dt.float32r
    Exp = mybir.ActivationFunctionType.Exp

    n_kv_pad = ((n_kv + P - 1) // P) * P          # 10880
    KT = n_kv_pad // P                            # 85 k-tiles
    last_valid = n_kv - (KT - 1) * P              # 48 valid rows in last k-tile
    scale = 1.0 / math.sqrt(d)

    # q-block layout within the per-core q tensor: [A (1440->1536 pad) | B (720->768 pad)]
    ablk = ((L + QBLK - 1) // QBLK) * QBLK        # 1536
    QT_N = ablk + QBLK                            # 2304
    # passes: (ka/va slot, q column offset)
    passes = [("a", 0), ("a", QBLK), ("b", ablk)]

    nc = bacc.Bacc(None, target_bir_lowering=False)

    qt_d = nc.dram_tensor("qt", [P, QT_N], f32r, kind="ExternalInput")
    qts_d = nc.dram_tensor("qts", [P, QT_N], f32, kind="ExternalInput")
    cosq_d = nc.dram_tensor("cosq", [P, QT_N], f32, kind="ExternalInput")
    sinq_d = nc.dram_tensor("sinq", [P, QT_N], f32, kind="ExternalInput")
    kt_d = {s: nc.dram_tensor(f"kt{s}", [P, n_kv_pad - n_cache], f32r,
                              kind="ExternalInput") for s in "ab"}
    kts_d = {s: nc.dram_tensor(f"kts{s}", [P, L], f32,
                               kind="ExternalInput") for s in "ab"}
    kc_d = {s: nc.dram_tensor(f"kc{s}", [P, n_cache], f32r,
                              kind="ExternalInput") for s in "ab"}
    va_d = {s: nc.dram_tensor(f"va{s}", [n_kv_pad, d], f32r,
                              kind="ExternalInput") for s in "ab"}
    # [128, 256]: cols 0:128 all-ones matrix, cols 128:256 rows<last_valid ones
    ones_d = nc.dram_tensor("onesm", [P, 2 * P], f32r, kind="ExternalInput")
    ident_d = nc.dram_tensor("ident", [P, P], f32, kind="ExternalInput")
    out_d = nc.dram_tensor("o", [QT_N, d], f32, kind="ExternalOutput")

    with tile.TileContext(nc) as tc:
        with tc.tile_pool(name="big", bufs=1) as big, \
             tc.tile_pool(name="work", bufs=2) as work, \
             tc.tile_pool(name="psum", bufs=1, space="PSUM") as psum:

            ident = big.tile([P, P], f32, tag="ident", name="ident")
            nc.sync.dma_start(ident[:], ident_d[:])
            onesm = big.tile([P, 2 * P], f32r, tag="onesm", name="onesm")
            nc.sync.dma_start(onesm[:], ones_d[:])

            cosq = big.tile([P, QT_N], f32, tag="cosq", name="cosq")
            sinq = big.tile([P, QT_N], f32, tag="sinq", name="sinq")
            nc.sync.dma_start(cosq[:], cosq_d[:])
            nc.sync.dma_start(sinq[:], sinq_d[:])

            rq = big.tile([P, QT_N], f32r, tag="rq", name="rq")
            ka = big.tile([P, n_kv_pad], f32r, tag="ka", name="ka")
            va = big.tile([P, n_kv_pad], f32r, tag="va", name="va")

            def rope(dst_f32r, src_f32r, swap_f32, n_cols, tab_off):
                """dst = rope(src) where swap_f32 holds the half-swapped copy
                (host-built); all operands lane-aligned [P, n_cols]."""
                src = src_f32r.bitcast(f32)
                C = cosq[:, tab_off:tab_off + n_cols]
                S = sinq[:, tab_off:tab_off + n_cols]
                t1 = work.tile([P, n_cols], f32, tag="ropet1", name="ropet1")
                t2 = work.tile([P, n_cols], f32, tag="ropet2", name="ropet2")
                nc.vector.tensor_mul(t1[:, :], swap_f32, S)  # [-sin;sin] folded
                nc.vector.tensor_mul(t2[:, :], src, C)
                nc.vector.tensor_add(dst_f32r, t2[:, :].bitcast(f32r),
                                     t1[:, :].bitcast(f32r))

            # --- q load + rope (both blocks); x lands in rq, swap staged ---
            nc.sync.dma_start(rq[:], qt_d[:])
            qsw = work.tile([P, QT_N], f32, tag="swstage", bufs=1, name="qsw")
            nc.sync.dma_start(qsw[:], qts_d[:])
            rope(rq[:, 0:ablk], rq[:, 0:ablk], qsw[:, 0:ablk], ablk, 0)
            rope(rq[:, ablk:QT_N], rq[:, ablk:QT_N], qsw[:, ablk:QT_N], QBLK,
                 ablk)

            def load_kv_slot(s):
                """DMA cache keys + new keys + values for slot s; rope new keys."""
                ncols = n_cache // 4
                for cidx in range(4):
                    lo = cidx * ncols
                    nc.sync.dma_start(ka[:, lo:lo + ncols],
                                      kc_d[s][:, lo:lo + ncols])
                nc.sync.dma_start(ka[:, n_cache:n_kv_pad], kt_d[s][:])
                ksw = work.tile([P, L], f32, tag="swstage", bufs=1,
                                name=f"ksw{s}")
                nc.sync.dma_start(ksw[:], kts_d[s][:])
                rope(ka[:, n_cache:n_cache + L], ka[:, n_cache:n_cache + L],
                     ksw[:, :], L, 0)
                # values: [n_kv_pad, d] rows -> [P, KT*d] tiles
                src = va_d[s][:].rearrange("(t p) d -> p t d", p=P)
                dst = va[:].rearrange("p (t d) -> p t d", d=d)
                qtr = KT // 4
                for cidx in range(4):
                    t0 = cidx * qtr
                    t1_ = KT if cidx == 3 else (cidx + 1) * qtr
                    nc.sync.dma_start(dst[:, t0:t1_, :], src[:, t0:t1_, :])

            load_kv_slot("a")

            # B-pass DVE softmax-denominator accumulators (ping-pong)
            sacc = [big.tile([P, QBLK], f32, tag=f"sacc{i}", name=f"sacc{i}")
                    for i in range(2)]

            def run_pass(pidx, slot, q0):
                is_b = slot == "b"
                ot_ps = psum.tile([P, QBLK], f32, tag="ot", name=f"ot{pidx}")
                sums_ps = psum.tile([P, QBLK], f32, tag="sums", name=f"sums{pidx}")
                for kt in range(KT):
                    ksl = ka[:, kt * P:(kt + 1) * P]
                    vsl = va[:, kt * d:(kt + 1) * d]
                    onemat = (onesm[:, P:2 * P] if kt == KT - 1
                              else onesm[:, 0:P])
                    sc = psum.tile([P, QBLK], f32, tag="sc", bufs=2,
                                   name=f"sc{pidx}_{kt}")
                    for (co, cw) in CHUNKS:
                        nc.tensor.matmul(sc[:, co:co + cw], ksl,
                                         rq[:, q0 + co:q0 + co + cw],
                                         start=True, stop=True)
                    pt = work.tile([P, QBLK], f32r, tag="pt", bufs=3,
                                   name=f"pt{pidx}_{kt}")
                    nc.scalar.activation(pt[:], sc[:, :], Exp, scale=scale)
                    first, last = kt == 0, kt == KT - 1
                    for ci, (co, cw) in enumerate(CHUNKS):
                        nc.tensor.matmul(ot_ps[:, co:co + cw], vsl,
                                         pt[:, co:co + cw],
                                         start=first, stop=last)
                    if not is_b:
                        for ci, (co, cw) in enumerate(CHUNKS):
                            nc.tensor.matmul(sums_ps[:, co:co + cw], onemat,
                                             pt[:, co:co + cw],
                                             start=first, stop=last)
                    else:
                        # denominators on DVE (frees TensorE); last k-tile via
                        # masked ones-matmul (pad rows must not count)
                        if kt == 0:
                            nc.vector.tensor_copy(sacc[0][:], pt[:].bitcast(f32))
                        elif kt < KT - 1:
                            nc.vector.tensor_add(sacc[kt % 2][:],
                                                 sacc[(kt + 1) % 2][:],
                                                 pt[:].bitcast(f32))
                        else:
                            # single f32r-typed producer for the matmul input
                            saccr = work.tile([P, QBLK], f32r, tag="saccr",
                                              bufs=1, name="saccr")
                            nc.vector.tensor_copy(saccr[:],
                                                  sacc[(kt - 1) % 2][:])
                            for ci, (co, cw) in enumerate(CHUNKS):
                                nc.tensor.matmul(sums_ps[:, co:co + cw],
                                                 onemat, pt[:, co:co + cw],
                                                 start=True, stop=False)
                                nc.tensor.matmul(
                                    sums_ps[:, co:co + cw], onesm[:, 0:P],
                                    saccr[:, co:co + cw],
                                    start=False, stop=True)

                # ---- drain: transpose + normalize + store ----
                ot_sb = work.tile([P, QBLK], f32, tag="otsb", name=f"otsb{pidx}")
                nc.vector.tensor_copy(ot_sb[:], ot_ps[:, :])
                # sums rows are all identical; keep lane 0
                s_sb = work.tile([1, QBLK], f32, tag="ssb", name=f"ssb{pidx}")
                nc.vector.tensor_copy(s_sb[0:1, :], sums_ps[0:1, :])
                for j in range(QBLK // P):
                    tp = psum.tile([P, P + 1], f32, tag="sc", bufs=2,
                                   name=f"tp{pidx}_{j}")
                    nc.tensor.transpose(tp[:, 0:P],
                                        ot_sb[:, j * P:(j + 1) * P], ident[:])
                    nc.tensor.transpose(tp[:, P:P + 1],
                                        s_sb[0:1, j * P:(j + 1) * P],
                                        ident[0:1, 0:1])
                    r_sb = work.tile([P, 1], f32, tag="rsb", name=f"rsb{pidx}_{j}")
                    nc.vector.reciprocal(r_sb[:], tp[:, P:P + 1])
                    o_sb = work.tile([P, d], f32, tag="osb", bufs=3,
                                     name=f"osb{pidx}_{j}")
                    nc.vector.tensor_scalar_mul(o_sb[:], tp[:, 0:P], r_sb[:])
                    row0 = q0 + j * P
                    nc.sync.dma_start(out_d[row0:row0 + P, :], o_sb[:])

            run_pass(0, "a", 0)
            run_pass(1, "a", QBLK)
            load_kv_slot("b")
            run_pass(2, "b", ablk)

    nc.finalize()
    meta = dict(QT_N=QT_N, ablk=ablk, n_kv_pad=n_kv_pad, last_valid=last_valid)
    return nc, meta


# ----------------------------------------------------------------------------
# host wrapper
# ----------------------------------------------------------------------------

def kernel(q, k, v, k_cache, v_cache, current_start, global_end_index,
           local_end_index, grid_f, grid_h, grid_w):
    from concourse.bass_utils import run_bass_kernel_spmd

    q = np.asarray(q, dtype=np.float32)
    k = np.asarray(k, dtype=np.float32)
    v = np.asarray(v, dtype=np.float32)
    k_cache = np.asarray(k_cache, dtype=np.float32)
    v_cache = np.asarray(v_cache, dtype=np.float32)
    current_start = int(current_start)
    global_end_index = int(global_end_index)
    local_end_index = int(local_end_index)
    grid_h, grid_w = int(grid_h), int(grid_w)

    B, L, n_heads, d = q.shape
    cache_len = k_cache.shape[1]
    frame_seqlen = grid_h * grid_w
    start_frame = current_start // frame_seqlen

    segs, local_end, kv_start = _plan_cache_segments(
        current_start, global_end_index, local_end_index, L, cache_len,
        frame_seqlen)
    n_cache = sum(hi - lo for lo, hi in segs)
    n_kv = n_cache + L

    key = (L, d, n_cache, n_kv)
    if key not in _BUILD_CACHE:
        _BUILD_CACHE[key] = _build_program(L, d, n_cache, n_kv)
    nc, meta = _BUILD_CACHE[key]
    QT_N, ablk, n_kv_pad = meta["QT_N"], meta["ablk"], meta["n_kv_pad"]
    last_valid = meta["last_valid"]

    # gather the cache window once (numpy)
    kc_full = np.concatenate([k_cache[0, lo:hi] for lo, hi in segs], axis=0)
    vc_full = np.concatenate([v_cache[0, lo:hi] for lo, hi in segs], axis=0)

    cos_t, sin_t = _rope_cos_sin(L, d, grid_h, grid_w, start_frame)  # [L, 64]
    H = d // 2
    perm = np.concatenate([np.arange(0, d, 2), np.arange(1, d, 2)])


    onesm = np.zeros((P, 2 * P), dtype=np.float32)
    onesm[:, 0:P] = 1.0
    onesm[0:last_valid, P:2 * P] = 1.0
    ident = np.eye(P, dtype=np.float32)

    perm_swap = np.concatenate([np.arange(1, d, 2), np.arange(0, d, 2)])

    def dei_T(x):  # [rows, d] -> de-interleaved transpose [d, rows]
        return np.ascontiguousarray(x.T[perm])

    def dei_T_swap(x):  # half-swapped variant: [odds; evens]
        return np.ascontiguousarray(x.T[perm_swap])

    half = L // 2
    n_pairs = n_heads // 3
    assert n_heads % 3 == 0 and n_pairs * 2 == 8, "sharding expects 12 heads/8 cores"

    in_maps = []
    core_heads = []
    for c in range(8):
        p, s = c // 2, c % 2
        headA = 3 * p if s == 0 else 3 * p + 2
        headB = 3 * p + 1
        qsl = slice(0, half) if s == 0 else slice(half, L)
        core_heads.append((headA, headB, qsl))

        cosq = np.ones((P, QT_N), dtype=np.float32)
        sinq = np.zeros((P, QT_N), dtype=np.float32)
        for (c0, tab) in ((0, slice(0, L)), (ablk, qsl)):
            ct, st = cos_t[tab].T, sin_t[tab].T
            w = ct.shape[1]
            cosq[0:H, c0:c0 + w] = ct
            cosq[H:P, c0:c0 + w] = ct
            sinq[0:H, c0:c0 + w] = -st
            sinq[H:P, c0:c0 + w] = st

        qt = np.zeros((P, QT_N), dtype=np.float32)
        qt[:, 0:L] = dei_T(q[0, :, headA, :])
        qt[:, ablk:ablk + half] = dei_T(q[0, qsl, headB, :])
        qts = np.zeros((P, QT_N), dtype=np.float32)
        qts[:, 0:L] = dei_T_swap(q[0, :, headA, :])
        qts[:, ablk:ablk + half] = dei_T_swap(q[0, qsl, headB, :])

        im = {"qt": qt, "qts": qts, "cosq": cosq, "sinq": sinq,
              "onesm": onesm, "ident": ident}
        for tag, h in (("a", headA), ("b", headB)):
            ktn = np.zeros((P, n_kv_pad - n_cache), dtype=np.float32)
            ktn[:, 0:L] = dei_T(k[0, :, h, :])
            im[f"kt{tag}"] = ktn
            im[f"kts{tag}"] = dei_T_swap(k[0, :, h, :])
            im[f"kc{tag}"] = dei_T(kc_full[:, h, :])
            vaa = np.zeros((n_kv_pad, d), dtype=np.float32)
            vaa[0:n_cache] = vc_full[:, h, :]
            vaa[n_cache:n_cache + L] = v[0, :, h, :]
            im[f"va{tag}"] = vaa
        in_maps.append(im)

    res = run_bass_kernel_spmd(nc, in_maps, core_ids=list(range(8)))

    out = np.empty((B, L, n_heads, d), dtype=np.float32)
    for c in range(8):
        headA, headB, qsl = core_heads[c]
        o = res.results[c]["o"]
        out[0, :, headA, :] = o[0:L]
        out[0, qsl, headB, :] = o[ablk:ablk + half]
    return out


# revision 14
# speedup vs baseline: 1.1936x; 1.1936x over previous
"""Trainium2 Bass kernel for windowed (sink/ring-buffer) self-attention with RoPE.

Contract: kernel(**inputs) takes FULL unsharded inputs (as produced by the
problem's setup_inputs) and returns the FULL output [B, L, n, d].

Sharding: 12 heads x 1440 queries are split across 8 NeuronCores as
1.5 "head-units" per core: each core owns one full head (1440 queries) plus
half of a head shared with its pair core (720 queries). All cores run the
same SPMD program on differently-sliced inputs.

Device program (per core):
  - RoPE applied on-chip to q and the new k block (4 tensor ops per block,
    using host-precomputed cos/sin tables in a de-interleaved d-layout that
    turns the rotation into plain elementwise ops; the d-permutation cancels
    inside the QK^T contraction).
  - S^T = ka^T q computed in [kv, q] orientation (fp32r matmuls), exp on
    ScalarE straight out of PSUM, then OT = va^T P and softmax denominators
    accumulated in PSUM; final transpose back to [q, d] on TensorE with a
    per-partition reciprocal scale.
"""

import math

import numpy as np

P = 128
THETA = 10000.0
LOCAL_ATTN_SIZE = 15
SINK_SIZE = 1

QBLK = 768          # q columns per pass (2 psum banks: 512 + 256 chunks)
CHUNKS = ((0, 512), (512, 256))

_BUILD_CACHE = {}


# ----------------------------------------------------------------------------
# host-side planning (mirrors the reference's python-int index logic)
# ----------------------------------------------------------------------------

def _plan_cache_segments(current_start, global_end_index, local_end_index,
                         num_new, cache_len, frame_seqlen):
    """Return (segments, local_end, kv_start): list of (lo, hi) slices of the
    ORIGINAL cache arrays that make up the pre-new-token part of the attention
    window, mirroring reference.py's roll/evict logic."""
    current_end = current_start + num_new
    sink_tokens = SINK_SIZE * frame_seqlen
    max_attn = LOCAL_ATTN_SIZE * frame_seqlen
    if current_end > global_end_index and num_new + local_end_index > cache_len:
        n_evict = num_new + local_end_index - cache_len
        n_roll = local_end_index - n_evict - sink_tokens
        local_end = local_end_index + current_end - global_end_index - n_evict
        roll_lo, roll_hi = sink_tokens, sink_tokens + n_roll

        def old_index(i):
            return i + n_evict if roll_lo <= i < roll_hi else i
    else:
        local_end = local_end_index + current_end - global_end_index
        n_evict = 0

        def old_index(i):
            return i

    local_start = local_end - num_new
    kv_start = max(0, local_end - max_attn)
    # contiguous segments of old_index over [kv_start, local_start)
    segs = []
    i = kv_start
    while i < local_start:
        lo = old_index(i)
        j = i
        while j + 1 < local_start and old_index(j + 1) == old_index(j) + 1:
            j += 1
        segs.append((lo, lo + (j - i + 1)))
        i = j + 1
    return segs, local_end, kv_start


def _rope_cos_sin(L, d, grid_h, grid_w, start_frame):
    """cos/sin angle tables [L, d//2] matching reference make_freqs/rope_apply."""
    c = d // 2
    d1 = d - 4 * (d // 6)
    d2 = 2 * (d // 6)
    inv1 = THETA ** (-(np.arange(0, d1, 2, dtype=np.float32) / np.float32(d1)))
    inv2 = THETA ** (-(np.arange(0, d2, 2, dtype=np.float32) / np.float32(d2)))
    inv3 = inv2
    hw = grid_h * grid_w
    pos = np.arange(L)
    f = pos // hw + start_frame
    hh = (pos % hw) // grid_w
    ww = pos % grid_w
    ang = np.concatenate([
        f[:, None].astype(np.float32) * inv1[None, :],
        hh[:, None].astype(np.float32) * inv2[None, :],
        ww[:, None].astype(np.float32) * inv3[None, :],
    ], axis=1)
    assert ang.shape == (L, c)
    return np.cos(ang).astype(np.float32), np.sin(ang).astype(np.float32)


# ----------------------------------------------------------------------------
# device program
# ----------------------------------------------------------------------------

def _build_program(L, d, n_cache, n_kv):
    """Build the SPMD Bass program for one core.

    L: new-token count (1440); d: head dim (128); n_cache: cache rows in the
    window (9360); n_kv: total kv rows (10800)."""
    import concourse.bass as bass
    import concourse.mybir as mybir
    import concourse.tile as tile
    from concourse import bacc

    f32 = mybir.dt.float32
    f32r = mybir.dt.float32r
    Exp = mybir.ActivationFunctionType.Exp

    n_kv_pad = ((n_kv + P - 1) // P) * P          # 10880
    KT = n_kv_pad // P                            # 85 k-tiles
    last_valid = n_kv - (KT - 1) * P              # 48 valid rows in last k-tile
    scale = 1.0 / math.sqrt(d)

    # q-block layout within the per-core q tensor: [A (1440->1536 pad) | B (720->768 pad)]
    ablk = ((L + QBLK - 1) // QBLK) * QBLK        # 1536
    QT_N = ablk + QBLK                            # 2304
    # passes: (ka/va slot, q column offset)
    passes = [("a", 0), ("a", QBLK), ("b", ablk)]

    nc = bacc.Bacc(None, target_bir_lowering=False)

    qt_d = nc.dram_tensor("qt", [P, QT_N], f32r, kind="ExternalInput")
    qts_d = nc.dram_tensor("qts", [P, QT_N], f32, kind="ExternalInput")
    cosq_d = nc.dram_tensor("cosq", [P, QT_N], f32, kind="ExternalInput")
    sinq_d = nc.dram_tensor("sinq", [P, QT_N], f32, kind="ExternalInput")
    kt_d = {s: nc.dram_tensor(f"kt{s}", [P, n_kv_pad - n_cache], f32r,
                              kind="ExternalInput") for s in "ab"}
    kts_d = {s: nc.dram_tensor(f"kts{s}", [P, L], f32,
                               kind="ExternalInput") for s in "ab"}
    kc_d = {s: nc.dram_tensor(f"kc{s}", [P, n_cache], f32r,
                              kind="ExternalInput") for s in "ab"}
    va_d = {s: nc.dram_tensor(f"va{s}", [n_kv_pad, d], f32r,
                              kind="ExternalInput") for s in "ab"}
    # [128, 256]: cols 0:128 all-ones matrix, cols 128:256 rows<last_valid ones
    ones_d = nc.dram_tensor("onesm", [P, 2 * P], f32r, kind="ExternalInput")
    ident_d = nc.dram_tensor("ident", [P, P], f32, kind="ExternalInput")
    out_d = nc.dram_tensor("o", [QT_N, d], f32, kind="ExternalOutput")

    with tile.TileContext(nc) as tc:
        with tc.tile_pool(name="big", bufs=1) as big, \
             tc.tile_pool(name="work", bufs=2) as work, \
             tc.tile_pool(name="psum", bufs=1, space="PSUM") as psum:

            ident = big.tile([P, P], f32, tag="ident", name="ident")
            nc.sync.dma_start(ident[:], ident_d[:])
            onesm = big.tile([P, 2 * P], f32r, tag="onesm", name="onesm")
            nc.sync.dma_start(onesm[:], ones_d[:])

            cosq = big.tile([P, QT_N], f32, tag="cosq", name="cosq")
            sinq = big.tile([P, QT_N], f32, tag="sinq", name="sinq")
            nc.sync.dma_start(cosq[:], cosq_d[:])
            nc.sync.dma_start(sinq[:], sinq_d[:])

            rq = big.tile([P, QT_N], f32r, tag="rq", name="rq")
            ka = big.tile([P, n_kv_pad], f32r, tag="ka", name="ka")
            va = big.tile([P, n_kv_pad], f32r, tag="va", name="va")

            def rope(dst_f32r, src_f32r, swap_f32, n_cols, tab_off):
                """dst = rope(src) where swap_f32 holds the half-swapped copy
                (host-built); all operands lane-aligned [P, n_cols]."""
                src = src_f32r.bitcast(f32)
                C = cosq[:, tab_off:tab_off + n_cols]
                S = sinq[:, tab_off:tab_off + n_cols]
                t1 = work.tile([P, n_cols], f32, tag="ropet1", name="ropet1")
                t2 = work.tile([P, n_cols], f32, tag="ropet2", name="ropet2")
                nc.vector.tensor_mul(t1[:, :], swap_f32, S)  # [-sin;sin] folded
                nc.vector.tensor_mul(t2[:, :], src, C)
                nc.vector.tensor_add(dst_f32r, t2[:, :].bitcast(f32r),
                                     t1[:, :].bitcast(f32r))

            # --- q load + rope (both blocks); x lands in rq, swap staged ---
            nc.sync.dma_start(rq[:], qt_d[:])
            qsw = work.tile([P, QT_N], f32, tag="swstage", bufs=1, name="qsw")
            nc.sync.dma_start(qsw[:], qts_d[:])
            rope(rq[:, 0:ablk], rq[:, 0:ablk], qsw[:, 0:ablk], ablk, 0)
            rope(rq[:, ablk:QT_N], rq[:, ablk:QT_N], qsw[:, ablk:QT_N], QBLK,
                 ablk)

            def load_kv_slot(s):
                """DMA cache keys + new keys + values for slot s; rope new keys."""
                ncols = n_cache // 4
                for cidx in range(4):
                    lo = cidx * ncols
                    nc.sync.dma_start(ka[:, lo:lo + ncols],
                                      kc_d[s][:, lo:lo + ncols])
                nc.sync.dma_start(ka[:, n_cache:n_kv_pad], kt_d[s][:])
                ksw = work.tile([P, L], f32, tag="swstage", bufs=1,
                                name=f"ksw{s}")
                nc.sync.dma_start(ksw[:], kts_d[s][:])
                rope(ka[:, n_cache:n_cache + L], ka[:, n_cache:n_cache + L],
                     ksw[:, :], L, 0)
                # values: [n_kv_pad, d] rows -> [P, KT*d] tiles
                src = va_d[s][:].rearrange("(t p) d -> p t d", p=P)
                dst = va[:].rearrange("p (t d) -> p t d", d=d)
                qtr = KT // 4
                for cidx in range(4):
                    t0 = cidx * qtr
                    t1_ = KT if cidx == 3 else (cidx + 1) * qtr
                    nc.sync.dma_start(dst[:, t0:t1_, :], src[:, t0:t1_, :])

            load_kv_slot("a")

            # B-pass DVE softmax-denominator accumulators (ping-pong)
            sacc = [big.tile([P, QBLK], f32, tag=f"sacc{i}", name=f"sacc{i}")
                    for i in range(2)]

            def run_pass(pidx, slot, q0, dve_frac=3):
                """One 768-wide q pass. Software-pipelined one k-tile deep:
                S^T(kt+1) is issued before AV/sums(kt) so TensorE never stalls
                on exp(kt). Softmax denominators: k-tiles with kt % 5 <
                dve_frac accumulate on DVE (ping-pong adds), the rest via
                ones-matmul on TensorE; both fold into sums_ps at pass end."""
                ot_ps = psum.tile([P, QBLK], f32, tag="ot", name=f"ot{pidx}")
                sums_ps = psum.tile([P, QBLK], f32, tag="sums", name=f"sums{pidx}")

                pts = {}
                state = dict(pe_first=True, n_dve=0)

                def st_mm(kt):
                    ksl = ka[:, kt * P:(kt + 1) * P]
                    sc = psum.tile([P, QBLK], f32, tag="sc", bufs=2,
                                   name=f"sc{pidx}_{kt}")
                    for (co, cw) in CHUNKS:
                        nc.tensor.matmul(sc[:, co:co + cw], ksl,
                                         rq[:, q0 + co:q0 + co + cw],
                                         start=True, stop=True)
                    pt = work.tile([P, QBLK], f32r, tag="pt", bufs=4,
                                   name=f"pt{pidx}_{kt}")
                    nc.scalar.activation(pt[:], sc[:, :], Exp, scale=scale)
                    pts[kt] = pt

                def av_sums(kt):
                    pt = pts.pop(kt)
                    vsl = va[:, kt * d:(kt + 1) * d]
                    first, last = kt == 0, kt == KT - 1
                    for (co, cw) in CHUNKS:
                        nc.tensor.matmul(ot_ps[:, co:co + cw], vsl,
                                         pt[:, co:co + cw],
                                         start=first, stop=last)
                    on_dve = kt % 5 < dve_frac and kt != KT - 1
                    if on_dve:
                        n = state["n_dve"]
                        if n == 0:
                            nc.vector.tensor_copy(sacc[0][:], pt[:].bitcast(f32))
                        else:
                            nc.vector.tensor_add(sacc[n % 2][:],
                                                 sacc[(n + 1) % 2][:],
                                                 pt[:].bitcast(f32))
                        state["n_dve"] = n + 1
                    else:
                        onemat = (onesm[:, P:2 * P] if kt == KT - 1
                                  else onesm[:, 0:P])
                        for (co, cw) in CHUNKS:
                            nc.tensor.matmul(sums_ps[:, co:co + cw], onemat,
                                             pt[:, co:co + cw],
                                             start=state["pe_first"], stop=False)
                        state["pe_first"] = False

                st_mm(0)
                for kt in range(KT):
                    if kt + 1 < KT:
                        st_mm(kt + 1)
                    av_sums(kt)
                # fold the DVE accumulator into sums_ps
                saccr = work.tile([P, QBLK], f32r, tag="saccr", bufs=1,
                                  name=f"saccr{pidx}")
                nc.vector.tensor_copy(saccr[:],
                                      sacc[(state["n_dve"] + 1) % 2][:])
                for (co, cw) in CHUNKS:
                    nc.tensor.matmul(sums_ps[:, co:co + cw], onesm[:, 0:P],
                                     saccr[:, co:co + cw],
                                     start=False, stop=True)

                # ---- drain: transpose + normalize + store ----
                ot_sb = work.tile([P, QBLK], f32, tag="otsb", name=f"otsb{pidx}")
                nc.vector.tensor_copy(ot_sb[:], ot_ps[:, :])
                # sums rows are all identical; keep lane 0
                s_sb = work.tile([1, QBLK], f32, tag="ssb", name=f"ssb{pidx}")
                nc.vector.tensor_copy(s_sb[0:1, :], sums_ps[0:1, :])
                for j in range(QBLK // P):
                    tp = psum.tile([P, P + 1], f32, tag="sc", bufs=2,
                                   name=f"tp{pidx}_{j}")
                    nc.tensor.transpose(tp[:, 0:P],
                                        ot_sb[:, j * P:(j + 1) * P], ident[:])
                    nc.tensor.transpose(tp[:, P:P + 1],
                                        s_sb[0:1, j * P:(j + 1) * P],
                                        ident[0:1, 0:1])
                    r_sb = work.tile([P, 1], f32, tag="rsb", name=f"rsb{pidx}_{j}")
                    nc.vector.reciprocal(r_sb[:], tp[:, P:P + 1])
                    o_sb = work.tile([P, d], f32, tag="osb", bufs=3,
                                     name=f"osb{pidx}_{j}")
                    nc.vector.tensor_scalar_mul(o_sb[:], tp[:, 0:P], r_sb[:])
                    row0 = q0 + j * P
                    nc.sync.dma_start(out_d[row0:row0 + P, :], o_sb[:])

            run_pass(0, "a", 0)
            run_pass(1, "a", QBLK)
            load_kv_slot("b")
            run_pass(2, "b", ablk)

    nc.finalize()
    meta = dict(QT_N=QT_N, ablk=ablk, n_kv_pad=n_kv_pad, last_valid=last_valid)
    return nc, meta


# ----------------------------------------------------------------------------
# host wrapper
# ----------------------------------------------------------------------------

def kernel(q, k, v, k_cache, v_cache, current_start, global_end_index,
           local_end_index, grid_f, grid_h, grid_w):
    from concourse.bass_utils import run_bass_kernel_spmd

    q = np.asarray(q, dtype=np.float32)
    k = np.asarray(k, dtype=np.float32)
    v = np.asarray(v, dtype=np.float32)
    k_cache = np.asarray(k_cache, dtype=np.float32)
    v_cache = np.asarray(v_cache, dtype=np.float32)
    current_start = int(current_start)
    global_end_index = int(global_end_index)
    local_end_index = int(local_end_index)
    grid_h, grid_w = int(grid_h), int(grid_w)

    B, L, n_heads, d = q.shape
    cache_len = k_cache.shape[1]
    frame_seqlen = grid_h * grid_w
    start_frame = current_start // frame_seqlen

    segs, local_end, kv_start = _plan_cache_segments(
        current_start, global_end_index, local_end_index, L, cache_len,
        frame_seqlen)
    n_cache = sum(hi - lo for lo, hi in segs)
    n_kv = n_cache + L

    key = (L, d, n_cache, n_kv)
    if key not in _BUILD_CACHE:
        _BUILD_CACHE[key] = _build_program(L, d, n_cache, n_kv)
    nc, meta = _BUILD_CACHE[key]
    QT_N, ablk, n_kv_pad = meta["QT_N"], meta["ablk"], meta["n_kv_pad"]
    last_valid = meta["last_valid"]

    # gather the cache window once (numpy)
    kc_full = np.concatenate([k_cache[0, lo:hi] for lo, hi in segs], axis=0)
    vc_full = np.concatenate([v_cache[0, lo:hi] for lo, hi in segs], axis=0)

    cos_t, sin_t = _rope_cos_sin(L, d, grid_h, grid_w, start_frame)  # [L, 64]
    H = d // 2
    perm = np.concatenate([np.arange(0, d, 2), np.arange(1, d, 2)])


    onesm = np.zeros((P, 2 * P), dtype=np.float32)
    onesm[:, 0:P] = 1.0
    onesm[0:last_valid, P:2 * P] = 1.0
    ident = np.eye(P, dtype=np.float32)

    perm_swap = np.concatenate([np.arange(1, d, 2), np.arange(0, d, 2)])

    def dei_T(x):  # [rows, d] -> de-interleaved transpose [d, rows]
        return np.ascontiguousarray(x.T[perm])

    def dei_T_swap(x):  # half-swapped variant: [odds; evens]
        return np.ascontiguousarray(x.T[perm_swap])

    half = L // 2
    n_pairs = n_heads // 3
    assert n_heads % 3 == 0 and n_pairs * 2 == 8, "sharding expects 12 heads/8 cores"

    in_maps = []
    core_heads = []
    for c in range(8):
        p, s = c // 2, c % 2
        headA = 3 * p if s == 0 else 3 * p + 2
        headB = 3 * p + 1
        qsl = slice(0, half) if s == 0 else slice(half, L)
        core_heads.append((headA, headB, qsl))

        cosq = np.ones((P, QT_N), dtype=np.float32)
        sinq = np.zeros((P, QT_N), dtype=np.float32)
        for (c0, tab) in ((0, slice(0, L)), (ablk, qsl)):
            ct, st = cos_t[tab].T, sin_t[tab].T
            w = ct.shape[1]
            cosq[0:H, c0:c0 + w] = ct
            cosq[H:P, c0:c0 + w] = ct
            sinq[0:H, c0:c0 + w] = -st
            sinq[H:P, c0:c0 + w] = st

        qt = np.zeros((P, QT_N), dtype=np.float32)
        qt[:, 0:L] = dei_T(q[0, :, headA, :])
        qt[:, ablk:ablk + half] = dei_T(q[0, qsl, headB, :])
        qts = np.zeros((P, QT_N), dtype=np.float32)
        qts[:, 0:L] = dei_T_swap(q[0, :, headA, :])
        qts[:, ablk:ablk + half] = dei_T_swap(q[0, qsl, headB, :])

        im = {"qt": qt, "qts": qts, "cosq": cosq, "sinq": sinq,
              "onesm": onesm, "ident": ident}
        for tag, h in (("a", headA), ("b", headB)):
            ktn = np.zeros((P, n_kv_pad - n_cache), dtype=np.float32)
            ktn[:, 0:L] = dei_T(k[0, :, h, :])
            im[f"kt{tag}"] = ktn
            im[f"kts{tag}"] = dei_T_swap(k[0, :, h, :])
            im[f"kc{tag}"] = dei_T(kc_full[:, h, :])
            vaa = np.zeros((n_kv_pad, d), dtype=np.float32)
            vaa[0:n_cache] = vc_full[:, h, :]
            vaa[n_cache:n_cache + L] = v[0, :, h, :]
            im[f"va{tag}"] = vaa
        in_maps.append(im)

    res = run_bass_kernel_spmd(nc, in_maps, core_ids=list(range(8)))

    out = np.empty((B, L, n_heads, d), dtype=np.float32)
    for c in range(8):
        headA, headB, qsl = core_heads[c]
        o = res.results[c]["o"]
        out[0, :, headA, :] = o[0:L]
        out[0, qsl, headB, :] = o[ablk:ablk + half]
    return out


# revision 15
# speedup vs baseline: 1.2508x; 1.0479x over previous
"""Trainium2 Bass kernel for windowed (sink/ring-buffer) self-attention with RoPE.

Contract: kernel(**inputs) takes FULL unsharded inputs (as produced by the
problem's setup_inputs) and returns the FULL output [B, L, n, d].

Sharding: 12 heads x 1440 queries are split across 8 NeuronCores as
1.5 "head-units" per core: each core owns one full head (1440 queries) plus
half of a head shared with its pair core (720 queries). All cores run the
same SPMD program on differently-sliced inputs.

Device program (per core):
  - RoPE applied on-chip to q and the new k block (4 tensor ops per block,
    using host-precomputed cos/sin tables in a de-interleaved d-layout that
    turns the rotation into plain elementwise ops; the d-permutation cancels
    inside the QK^T contraction).
  - S^T = ka^T q computed in [kv, q] orientation (fp32r matmuls), exp on
    ScalarE straight out of PSUM, then OT = va^T P and softmax denominators
    accumulated in PSUM; final transpose back to [q, d] on TensorE with a
    per-partition reciprocal scale.
"""

import math

import numpy as np

P = 128
THETA = 10000.0
LOCAL_ATTN_SIZE = 15
SINK_SIZE = 1

QBLK = 768          # q columns per pass (2 psum banks: 512 + 256 chunks)
CHUNKS = ((0, 512), (512, 256))

_BUILD_CACHE = {}


# ----------------------------------------------------------------------------
# host-side planning (mirrors the reference's python-int index logic)
# ----------------------------------------------------------------------------

def _plan_cache_segments(current_start, global_end_index, local_end_index,
                         num_new, cache_len, frame_seqlen):
    """Return (segments, local_end, kv_start): list of (lo, hi) slices of the
    ORIGINAL cache arrays that make up the pre-new-token part of the attention
    window, mirroring reference.py's roll/evict logic."""
    current_end = current_start + num_new
    sink_tokens = SINK_SIZE * frame_seqlen
    max_attn = LOCAL_ATTN_SIZE * frame_seqlen
    if current_end > global_end_index and num_new + local_end_index > cache_len:
        n_evict = num_new + local_end_index - cache_len
        n_roll = local_end_index - n_evict - sink_tokens
        local_end = local_end_index + current_end - global_end_index - n_evict
        roll_lo, roll_hi = sink_tokens, sink_tokens + n_roll

        def old_index(i):
            return i + n_evict if roll_lo <= i < roll_hi else i
    else:
        local_end = local_end_index + current_end - global_end_index
        n_evict = 0

        def old_index(i):
            return i

    local_start = local_end - num_new
    kv_start = max(0, local_end - max_attn)
    # contiguous segments of old_index over [kv_start, local_start)
    segs = []
    i = kv_start
    while i < local_start:
        lo = old_index(i)
        j = i
        while j + 1 < local_start and old_index(j + 1) == old_index(j) + 1:
            j += 1
        segs.append((lo, lo + (j - i + 1)))
        i = j + 1
    return segs, local_end, kv_start


def _rope_cos_sin(L, d, grid_h, grid_w, start_frame):
    """cos/sin angle tables [L, d//2] matching reference make_freqs/rope_apply."""
    c = d // 2
    d1 = d - 4 * (d // 6)
    d2 = 2 * (d // 6)
    inv1 = THETA ** (-(np.arange(0, d1, 2, dtype=np.float32) / np.float32(d1)))
    inv2 = THETA ** (-(np.arange(0, d2, 2, dtype=np.float32) / np.float32(d2)))
    inv3 = inv2
    hw = grid_h * grid_w
    pos = np.arange(L)
    f = pos // hw + start_frame
    hh = (pos % hw) // grid_w
    ww = pos % grid_w
    ang = np.concatenate([
        f[:, None].astype(np.float32) * inv1[None, :],
        hh[:, None].astype(np.float32) * inv2[None, :],
        ww[:, None].astype(np.float32) * inv3[None, :],
    ], axis=1)
    assert ang.shape == (L, c)
    return np.cos(ang).astype(np.float32), np.sin(ang).astype(np.float32)


# ----------------------------------------------------------------------------
# device program
# ----------------------------------------------------------------------------

def _build_program(L, d, n_cache, n_kv):
    """Build the SPMD Bass program for one core.

    L: new-token count (1440); d: head dim (128); n_cache: cache rows in the
    window (9360); n_kv: total kv rows (10800)."""
    import concourse.bass as bass
    import concourse.mybir as mybir
    import concourse.tile as tile
    from concourse import bacc

    f32 = mybir.dt.float32
    f32r = mybir.dt.float32r
    Exp = mybir.ActivationFunctionType.Exp

    n_kv_pad = ((n_kv + P - 1) // P) * P          # 10880
    KT = n_kv_pad // P                            # 85 k-tiles
    last_valid = n_kv - (KT - 1) * P              # 48 valid rows in last k-tile
    scale = 1.0 / math.sqrt(d)

    # q-block layout within the per-core q tensor: [A (1440->1536 pad) | B (720->768 pad)]
    ablk = ((L + QBLK - 1) // QBLK) * QBLK        # 1536
    QT_N = ablk + QBLK                            # 2304
    # passes: (ka/va slot, q column offset)
    passes = [("a", 0), ("a", QBLK), ("b", ablk)]

    nc = bacc.Bacc(None, target_bir_lowering=False)

    qt_d = nc.dram_tensor("qt", [P, QT_N], f32r, kind="ExternalInput")
    qts_d = nc.dram_tensor("qts", [P, QT_N], f32, kind="ExternalInput")
    cosq_d = nc.dram_tensor("cosq", [P, QT_N], f32, kind="ExternalInput")
    sinq_d = nc.dram_tensor("sinq", [P, QT_N], f32, kind="ExternalInput")
    kt_d = {s: nc.dram_tensor(f"kt{s}", [P, n_kv_pad - n_cache], f32r,
                              kind="ExternalInput") for s in "ab"}
    kts_d = {s: nc.dram_tensor(f"kts{s}", [P, L], f32,
                               kind="ExternalInput") for s in "ab"}
    kc_d = {s: nc.dram_tensor(f"kc{s}", [P, n_cache], f32r,
                              kind="ExternalInput") for s in "ab"}
    va_d = {s: nc.dram_tensor(f"va{s}", [n_kv_pad, d], f32r,
                              kind="ExternalInput") for s in "ab"}
    # [128, 256]: cols 0:128 all-ones matrix, cols 128:256 rows<last_valid ones
    ones_d = nc.dram_tensor("onesm", [P, 2 * P], f32r, kind="ExternalInput")
    ident_d = nc.dram_tensor("ident", [P, P], f32, kind="ExternalInput")
    out_d = nc.dram_tensor("o", [QT_N, d], f32, kind="ExternalOutput")

    with tile.TileContext(nc) as tc:
        with tc.tile_pool(name="big", bufs=1) as big, \
             tc.tile_pool(name="work", bufs=2) as work, \
             tc.tile_pool(name="psum", bufs=1, space="PSUM") as psum:

            ident = big.tile([P, P], f32, tag="ident", name="ident")
            nc.sync.dma_start(ident[:], ident_d[:])
            onesm = big.tile([P, 2 * P], f32r, tag="onesm", name="onesm")
            nc.sync.dma_start(onesm[:], ones_d[:])

            cosq = big.tile([P, QT_N], f32, tag="cosq", name="cosq")
            sinq = big.tile([P, QT_N], f32, tag="sinq", name="sinq")
            nc.sync.dma_start(cosq[:], cosq_d[:])
            nc.sync.dma_start(sinq[:], sinq_d[:])

            rq = big.tile([P, QT_N], f32r, tag="rq", name="rq")
            ka = big.tile([P, n_kv_pad], f32r, tag="ka", name="ka")
            va = big.tile([P, n_kv_pad], f32r, tag="va", name="va")

            def rope(dst_f32r, src_f32r, swap_f32, n_cols, tab_off):
                """dst = rope(src) where swap_f32 holds the half-swapped copy
                (host-built); all operands lane-aligned [P, n_cols]. Runs in
                <=QBLK column chunks so downstream matmuls unblock early."""
                for c0 in range(0, n_cols, QBLK):
                    w = min(QBLK, n_cols - c0)
                    src = src_f32r[:, c0:c0 + w].bitcast(f32)
                    C = cosq[:, tab_off + c0:tab_off + c0 + w]
                    S = sinq[:, tab_off + c0:tab_off + c0 + w]
                    t1 = work.tile([P, w], f32, tag="ropet1", name="ropet1")
                    t2 = work.tile([P, w], f32, tag="ropet2", name="ropet2")
                    nc.vector.tensor_mul(t1[:, :], swap_f32[:, c0:c0 + w], S)
                    nc.vector.tensor_mul(t2[:, :], src, C)
                    nc.vector.tensor_add(dst_f32r[:, c0:c0 + w],
                                         t2[:, :].bitcast(f32r),
                                         t1[:, :].bitcast(f32r))

            # --- q load + rope (both blocks); x lands in rq, swap staged ---
            nc.sync.dma_start(rq[:], qt_d[:])
            qsw = work.tile([P, QT_N], f32, tag="swstage", bufs=1, name="qsw")
            nc.sync.dma_start(qsw[:], qts_d[:])
            rope(rq[:, 0:ablk], rq[:, 0:ablk], qsw[:, 0:ablk], ablk, 0)
            rope(rq[:, ablk:QT_N], rq[:, ablk:QT_N], qsw[:, ablk:QT_N], QBLK,
                 ablk)

            def load_kv_slot(s):
                """DMA cache keys + new keys + values for slot s; rope new keys."""
                nch = 8
                ncols = n_cache // nch
                for cidx in range(nch):
                    lo = cidx * ncols
                    hi = n_cache if cidx == nch - 1 else lo + ncols
                    nc.sync.dma_start(ka[:, lo:hi], kc_d[s][:, lo:hi])
                nc.sync.dma_start(ka[:, n_cache:n_kv_pad], kt_d[s][:])
                ksw = work.tile([P, L], f32, tag="swstage", bufs=1,
                                name=f"ksw{s}")
                nc.sync.dma_start(ksw[:], kts_d[s][:])
                rope(ka[:, n_cache:n_cache + L], ka[:, n_cache:n_cache + L],
                     ksw[:, :], L, 0)
                # values: [n_kv_pad, d] rows -> [P, KT*d] tiles
                src = va_d[s][:].rearrange("(t p) d -> p t d", p=P)
                dst = va[:].rearrange("p (t d) -> p t d", d=d)
                qtr = KT // 8
                for cidx in range(8):
                    t0 = cidx * qtr
                    t1_ = KT if cidx == 7 else (cidx + 1) * qtr
                    nc.sync.dma_start(dst[:, t0:t1_, :], src[:, t0:t1_, :])

            load_kv_slot("a")

            # B-pass DVE softmax-denominator accumulators (ping-pong)
            sacc = [big.tile([P, QBLK], f32, tag=f"sacc{i}", name=f"sacc{i}")
                    for i in range(2)]

            def run_pass(pidx, slot, q0, dve_frac=4):
                """One 768-wide q pass. Software-pipelined one k-tile deep:
                S^T(kt+1) is issued before AV/sums(kt) so TensorE never stalls
                on exp(kt). Softmax denominators: k-tiles with kt % 5 <
                dve_frac accumulate on DVE (ping-pong adds), the rest via
                ones-matmul on TensorE; both fold into sums_ps at pass end."""
                ot_ps = psum.tile([P, QBLK], f32, tag="ot", name=f"ot{pidx}")
                sums_ps = psum.tile([P, QBLK], f32, tag="sums", name=f"sums{pidx}")

                pts = {}
                state = dict(pe_first=True, n_dve=0)

                def st_mm(kt):
                    ksl = ka[:, kt * P:(kt + 1) * P]
                    sc = psum.tile([P, QBLK], f32, tag="sc", bufs=2,
                                   name=f"sc{pidx}_{kt}")
                    for (co, cw) in CHUNKS:
                        nc.tensor.matmul(sc[:, co:co + cw], ksl,
                                         rq[:, q0 + co:q0 + co + cw],
                                         start=True, stop=True)
                    pt = work.tile([P, QBLK], f32r, tag="pt", bufs=4,
                                   name=f"pt{pidx}_{kt}")
                    nc.scalar.activation(pt[:], sc[:, :], Exp, scale=scale)
                    pts[kt] = pt

                def av_sums(kt):
                    pt = pts.pop(kt)
                    vsl = va[:, kt * d:(kt + 1) * d]
                    first, last = kt == 0, kt == KT - 1
                    for (co, cw) in CHUNKS:
                        nc.tensor.matmul(ot_ps[:, co:co + cw], vsl,
                                         pt[:, co:co + cw],
                                         start=first, stop=last)
                    on_dve = kt % 5 < dve_frac and kt != KT - 1
                    if on_dve:
                        n = state["n_dve"]
                        if n == 0:
                            nc.vector.tensor_copy(sacc[0][:], pt[:].bitcast(f32))
                        else:
                            nc.vector.tensor_add(sacc[n % 2][:],
                                                 sacc[(n + 1) % 2][:],
                                                 pt[:].bitcast(f32))
                        state["n_dve"] = n + 1
                    else:
                        onemat = (onesm[:, P:2 * P] if kt == KT - 1
                                  else onesm[:, 0:P])
                        for (co, cw) in CHUNKS:
                            nc.tensor.matmul(sums_ps[:, co:co + cw], onemat,
                                             pt[:, co:co + cw],
                                             start=state["pe_first"], stop=False)
                        state["pe_first"] = False

                st_mm(0)
                for kt in range(KT):
                    if kt + 1 < KT:
                        st_mm(kt + 1)
                    av_sums(kt)
                # fold the DVE accumulator into sums_ps
                saccr = work.tile([P, QBLK], f32r, tag="saccr", bufs=1,
                                  name=f"saccr{pidx}")
                nc.vector.tensor_copy(saccr[:],
                                      sacc[(state["n_dve"] + 1) % 2][:])
                for (co, cw) in CHUNKS:
                    nc.tensor.matmul(sums_ps[:, co:co + cw], onesm[:, 0:P],
                                     saccr[:, co:co + cw],
                                     start=False, stop=True)

                # ---- drain: transpose + normalize + store ----
                ot_sb = work.tile([P, QBLK], f32, tag="otsb", name=f"otsb{pidx}")
                nc.vector.tensor_copy(ot_sb[:], ot_ps[:, :])
                # sums rows are all identical; keep lane 0
                s_sb = work.tile([1, QBLK], f32, tag="ssb", name=f"ssb{pidx}")
                nc.vector.tensor_copy(s_sb[0:1, :], sums_ps[0:1, :])
                for j in range(QBLK // P):
                    tp = psum.tile([P, P + 1], f32, tag="sc", bufs=2,
                                   name=f"tp{pidx}_{j}")
                    nc.tensor.transpose(tp[:, 0:P],
                                        ot_sb[:, j * P:(j + 1) * P], ident[:])
                    nc.tensor.transpose(tp[:, P:P + 1],
                                        s_sb[0:1, j * P:(j + 1) * P],
                                        ident[0:1, 0:1])
                    r_sb = work.tile([P, 1], f32, tag="rsb", name=f"rsb{pidx}_{j}")
                    nc.vector.reciprocal(r_sb[:], tp[:, P:P + 1])
                    o_sb = work.tile([P, d], f32, tag="osb", bufs=3,
                                     name=f"osb{pidx}_{j}")
                    nc.vector.tensor_scalar_mul(o_sb[:], tp[:, 0:P], r_sb[:])
                    row0 = q0 + j * P
                    nc.sync.dma_start(out_d[row0:row0 + P, :], o_sb[:])

            run_pass(0, "a", 0)
            run_pass(1, "a", QBLK)
            load_kv_slot("b")
            run_pass(2, "b", ablk)

    nc.finalize()
    meta = dict(QT_N=QT_N, ablk=ablk, n_kv_pad=n_kv_pad, last_valid=last_valid)
    return nc, meta


# ----------------------------------------------------------------------------
# host wrapper
# ----------------------------------------------------------------------------

def kernel(q, k, v, k_cache, v_cache, current_start, global_end_index,
           local_end_index, grid_f, grid_h, grid_w):
    from concourse.bass_utils import run_bass_kernel_spmd

    q = np.asarray(q, dtype=np.float32)
    k = np.asarray(k, dtype=np.float32)
    v = np.asarray(v, dtype=np.float32)
    k_cache = np.asarray(k_cache, dtype=np.float32)
    v_cache = np.asarray(v_cache, dtype=np.float32)
    current_start = int(current_start)
    global_end_index = int(global_end_index)
    local_end_index = int(local_end_index)
    grid_h, grid_w = int(grid_h), int(grid_w)

    B, L, n_heads, d = q.shape
    cache_len = k_cache.shape[1]
    frame_seqlen = grid_h * grid_w
    start_frame = current_start // frame_seqlen

    segs, local_end, kv_start = _plan_cache_segments(
        current_start, global_end_index, local_end_index, L, cache_len,
        frame_seqlen)
    n_cache = sum(hi - lo for lo, hi in segs)
    n_kv = n_cache + L

    key = (L, d, n_cache, n_kv)
    if key not in _BUILD_CACHE:
        _BUILD_CACHE[key] = _build_program(L, d, n_cache, n_kv)
    nc, meta = _BUILD_CACHE[key]
    QT_N, ablk, n_kv_pad = meta["QT_N"], meta["ablk"], meta["n_kv_pad"]
    last_valid = meta["last_valid"]

    # gather the cache window once (numpy)
    kc_full = np.concatenate([k_cache[0, lo:hi] for lo, hi in segs], axis=0)
    vc_full = np.concatenate([v_cache[0, lo:hi] for lo, hi in segs], axis=0)

    cos_t, sin_t = _rope_cos_sin(L, d, grid_h, grid_w, start_frame)  # [L, 64]
    H = d // 2
    perm = np.concatenate([np.arange(0, d, 2), np.arange(1, d, 2)])


    onesm = np.zeros((P, 2 * P), dtype=np.float32)
    onesm[:, 0:P] = 1.0
    onesm[0:last_valid, P:2 * P] = 1.0
    ident = np.eye(P, dtype=np.float32)

    perm_swap = np.concatenate([np.arange(1, d, 2), np.arange(0, d, 2)])

    def dei_T(x):  # [rows, d] -> de-interleaved transpose [d, rows]
        return np.ascontiguousarray(x.T[perm])

    def dei_T_swap(x):  # half-swapped variant: [odds; evens]
        return np.ascontiguousarray(x.T[perm_swap])

    half = L // 2
    n_pairs = n_heads // 3
    assert n_heads % 3 == 0 and n_pairs * 2 == 8, "sharding expects 12 heads/8 cores"

    in_maps = []
    core_heads = []
    for c in range(8):
        p, s = c // 2, c % 2
        headA = 3 * p if s == 0 else 3 * p + 2
        headB = 3 * p + 1
        qsl = slice(0, half) if s == 0 else slice(half, L)
        core_heads.append((headA, headB, qsl))

        cosq = np.ones((P, QT_N), dtype=np.float32)
        sinq = np.zeros((P, QT_N), dtype=np.float32)
        for (c0, tab) in ((0, slice(0, L)), (ablk, qsl)):
            ct, st = cos_t[tab].T, sin_t[tab].T
            w = ct.shape[1]
            cosq[0:H, c0:c0 + w] = ct
            cosq[H:P, c0:c0 + w] = ct
            sinq[0:H, c0:c0 + w] = -st
            sinq[H:P, c0:c0 + w] = st

        qt = np.zeros((P, QT_N), dtype=np.float32)
        qt[:, 0:L] = dei_T(q[0, :, headA, :])
        qt[:, ablk:ablk + half] = dei_T(q[0, qsl, headB, :])
        qts = np.zeros((P, QT_N), dtype=np.float32)
        qts[:, 0:L] = dei_T_swap(q[0, :, headA, :])
        qts[:, ablk:ablk + half] = dei_T_swap(q[0, qsl, headB, :])

        im = {"qt": qt, "qts": qts, "cosq": cosq, "sinq": sinq,
              "onesm": onesm, "ident": ident}
        for tag, h in (("a", headA), ("b", headB)):
            ktn = np.zeros((P, n_kv_pad - n_cache), dtype=np.float32)
            ktn[:, 0:L] = dei_T(k[0, :, h, :])
            im[f"kt{tag}"] = ktn
            im[f"kts{tag}"] = dei_T_swap(k[0, :, h, :])
            im[f"kc{tag}"] = dei_T(kc_full[:, h, :])
            vaa = np.zeros((n_kv_pad, d), dtype=np.float32)
            vaa[0:n_cache] = vc_full[:, h, :]
            vaa[n_cache:n_cache + L] = v[0, :, h, :]
            im[f"va{tag}"] = vaa
        in_maps.append(im)

    res = run_bass_kernel_spmd(nc, in_maps, core_ids=list(range(8)))

    out = np.empty((B, L, n_heads, d), dtype=np.float32)
    for c in range(8):
        headA, headB, qsl = core_heads[c]
        o = res.results[c]["o"]
        out[0, :, headA, :] = o[0:L]
        out[0, qsl, headB, :] = o[ablk:ablk + half]
    return out


# revision 16
# speedup vs baseline: 1.2651x; 1.0115x over previous
"""Trainium2 Bass kernel for windowed (sink/ring-buffer) self-attention with RoPE.

Contract: kernel(**inputs) takes FULL unsharded inputs (as produced by the
problem's setup_inputs) and returns the FULL output [B, L, n, d].

Sharding: 12 heads x 1440 queries are split across 8 NeuronCores as
1.5 "head-units" per core: each core owns one full head (1440 queries) plus
half of a head shared with its pair core (720 queries). All cores run the
same SPMD program on differently-sliced inputs.

Device program (per core):
  - RoPE applied on-chip to q and the new k block (4 tensor ops per block,
    using host-precomputed cos/sin tables in a de-interleaved d-layout that
    turns the rotation into plain elementwise ops; the d-permutation cancels
    inside the QK^T contraction).
  - S^T = ka^T q computed in [kv, q] orientation (fp32r matmuls), exp on
    ScalarE straight out of PSUM, then OT = va^T P and softmax denominators
    accumulated in PSUM; final transpose back to [q, d] on TensorE with a
    per-partition reciprocal scale.
"""

import math

import numpy as np

P = 128
THETA = 10000.0
LOCAL_ATTN_SIZE = 15
SINK_SIZE = 1

QBLK = 768          # q columns per pass (2 psum banks: 512 + 256 chunks)
CHUNKS = ((0, 512), (512, 256))

_BUILD_CACHE = {}


# ----------------------------------------------------------------------------
# host-side planning (mirrors the reference's python-int index logic)
# ----------------------------------------------------------------------------

def _plan_cache_segments(current_start, global_end_index, local_end_index,
                         num_new, cache_len, frame_seqlen):
    """Return (segments, local_end, kv_start): list of (lo, hi) slices of the
    ORIGINAL cache arrays that make up the pre-new-token part of the attention
    window, mirroring reference.py's roll/evict logic."""
    current_end = current_start + num_new
    sink_tokens = SINK_SIZE * frame_seqlen
    max_attn = LOCAL_ATTN_SIZE * frame_seqlen
    if current_end > global_end_index and num_new + local_end_index > cache_len:
        n_evict = num_new + local_end_index - cache_len
        n_roll = local_end_index - n_evict - sink_tokens
        local_end = local_end_index + current_end - global_end_index - n_evict
        roll_lo, roll_hi = sink_tokens, sink_tokens + n_roll

        def old_index(i):
            return i + n_evict if roll_lo <= i < roll_hi else i
    else:
        local_end = local_end_index + current_end - global_end_index
        n_evict = 0

        def old_index(i):
            return i

    local_start = local_end - num_new
    kv_start = max(0, local_end - max_attn)
    # contiguous segments of old_index over [kv_start, local_start)
    segs = []
    i = kv_start
    while i < local_start:
        lo = old_index(i)
        j = i
        while j + 1 < local_start and old_index(j + 1) == old_index(j) + 1:
            j += 1
        segs.append((lo, lo + (j - i + 1)))
        i = j + 1
    return segs, local_end, kv_start


def _rope_cos_sin(L, d, grid_h, grid_w, start_frame):
    """cos/sin angle tables [L, d//2] matching reference make_freqs/rope_apply."""
    c = d // 2
    d1 = d - 4 * (d // 6)
    d2 = 2 * (d // 6)
    inv1 = THETA ** (-(np.arange(0, d1, 2, dtype=np.float32) / np.float32(d1)))
    inv2 = THETA ** (-(np.arange(0, d2, 2, dtype=np.float32) / np.float32(d2)))
    inv3 = inv2
    hw = grid_h * grid_w
    pos = np.arange(L)
    f = pos // hw + start_frame
    hh = (pos % hw) // grid_w
    ww = pos % grid_w
    ang = np.concatenate([
        f[:, None].astype(np.float32) * inv1[None, :],
        hh[:, None].astype(np.float32) * inv2[None, :],
        ww[:, None].astype(np.float32) * inv3[None, :],
    ], axis=1)
    assert ang.shape == (L, c)
    return np.cos(ang).astype(np.float32), np.sin(ang).astype(np.float32)


# ----------------------------------------------------------------------------
# device program
# ----------------------------------------------------------------------------

def _build_program(L, d, n_cache, n_kv):
    """Build the SPMD Bass program for one core.

    L: new-token count (1440); d: head dim (128); n_cache: cache rows in the
    window (9360); n_kv: total kv rows (10800)."""
    import concourse.bass as bass
    import concourse.mybir as mybir
    import concourse.tile as tile
    from concourse import bacc

    f32 = mybir.dt.float32
    f32r = mybir.dt.float32r
    Exp = mybir.ActivationFunctionType.Exp

    n_kv_pad = ((n_kv + P - 1) // P) * P          # 10880
    KT = n_kv_pad // P                            # 85 k-tiles
    last_valid = n_kv - (KT - 1) * P              # 48 valid rows in last k-tile
    scale = 1.0 / math.sqrt(d)

    # q-block layout within the per-core q tensor: [A (1440->1536 pad) | B (720->768 pad)]
    ablk = ((L + QBLK - 1) // QBLK) * QBLK        # 1536
    QT_N = ablk + QBLK                            # 2304
    # passes: (ka/va slot, q column offset)
    passes = [("a", 0), ("a", QBLK), ("b", ablk)]

    nc = bacc.Bacc(None, target_bir_lowering=False)

    qt_d = nc.dram_tensor("qt", [P, QT_N], f32r, kind="ExternalInput")
    qts_d = nc.dram_tensor("qts", [P, QT_N], f32, kind="ExternalInput")
    cosq_d = nc.dram_tensor("cosq", [P, QT_N], f32, kind="ExternalInput")
    sinq_d = nc.dram_tensor("sinq", [P, QT_N], f32, kind="ExternalInput")
    kt_d = {s: nc.dram_tensor(f"kt{s}", [P, n_kv_pad - n_cache], f32r,
                              kind="ExternalInput") for s in "ab"}
    kts_d = {s: nc.dram_tensor(f"kts{s}", [P, L], f32,
                               kind="ExternalInput") for s in "ab"}
    kc_d = {s: nc.dram_tensor(f"kc{s}", [P, n_cache], f32r,
                              kind="ExternalInput") for s in "ab"}
    va_d = {s: nc.dram_tensor(f"va{s}", [n_kv_pad, d], f32r,
                              kind="ExternalInput") for s in "ab"}
    # [128, 256]: cols 0:128 all-ones matrix, cols 128:256 rows<last_valid ones
    ones_d = nc.dram_tensor("onesm", [P, 2 * P], f32r, kind="ExternalInput")
    ident_d = nc.dram_tensor("ident", [P, P], f32, kind="ExternalInput")
    out_d = nc.dram_tensor("o", [QT_N, d], f32, kind="ExternalOutput")

    with tile.TileContext(nc) as tc:
        with tc.tile_pool(name="big", bufs=1) as big, \
             tc.tile_pool(name="work", bufs=2) as work, \
             tc.tile_pool(name="psum", bufs=1, space="PSUM") as psum:

            ident = big.tile([P, P], f32, tag="ident", name="ident")
            nc.sync.dma_start(ident[:], ident_d[:])
            onesm = big.tile([P, 2 * P], f32r, tag="onesm", name="onesm")
            nc.sync.dma_start(onesm[:], ones_d[:])

            cosq = big.tile([P, QT_N], f32, tag="cosq", name="cosq")
            sinq = big.tile([P, QT_N], f32, tag="sinq", name="sinq")

            rq = big.tile([P, QT_N], f32r, tag="rq", name="rq")
            ka = big.tile([P, n_kv_pad], f32r, tag="ka", name="ka")
            va = big.tile([P, n_kv_pad], f32r, tag="va", name="va")

            def rope(dst_f32r, src_f32r, swap_f32, n_cols, tab_off):
                """dst = rope(src) where swap_f32 holds the half-swapped copy
                (host-built); all operands lane-aligned [P, n_cols]. Runs in
                <=QBLK column chunks so downstream matmuls unblock early."""
                for c0 in range(0, n_cols, QBLK):
                    w = min(QBLK, n_cols - c0)
                    src = src_f32r[:, c0:c0 + w].bitcast(f32)
                    C = cosq[:, tab_off + c0:tab_off + c0 + w]
                    S = sinq[:, tab_off + c0:tab_off + c0 + w]
                    t1 = work.tile([P, w], f32, tag="ropet1", name="ropet1")
                    t2 = work.tile([P, w], f32, tag="ropet2", name="ropet2")
                    nc.vector.tensor_mul(t1[:, :], swap_f32[:, c0:c0 + w], S)
                    nc.vector.tensor_mul(t2[:, :], src, C)
                    nc.vector.tensor_add(dst_f32r[:, c0:c0 + w],
                                         t2[:, :].bitcast(f32r),
                                         t1[:, :].bitcast(f32r))

            # --- q load + rope (both blocks); x lands in rq, swap staged ---
            # chunked so the first rope chunk (and first matmul) starts early
            qsw = work.tile([P, QT_N], f32, tag="swstage", bufs=1, name="qsw")
            for c0 in range(0, QT_N, QBLK):
                c1 = c0 + QBLK
                nc.sync.dma_start(rq[:, c0:c1], qt_d[:, c0:c1])
                nc.sync.dma_start(qsw[:, c0:c1], qts_d[:, c0:c1])
                nc.sync.dma_start(cosq[:, c0:c1], cosq_d[:, c0:c1])
                nc.sync.dma_start(sinq[:, c0:c1], sinq_d[:, c0:c1])
            rope(rq[:, 0:ablk], rq[:, 0:ablk], qsw[:, 0:ablk], ablk, 0)
            rope(rq[:, ablk:QT_N], rq[:, ablk:QT_N], qsw[:, ablk:QT_N], QBLK,
                 ablk)

            def load_kv_slot(s):
                """DMA cache keys + new keys + values for slot s; rope new keys."""
                nch = 8
                ncols = n_cache // nch
                for cidx in range(nch):
                    lo = cidx * ncols
                    hi = n_cache if cidx == nch - 1 else lo + ncols
                    nc.sync.dma_start(ka[:, lo:hi], kc_d[s][:, lo:hi])
                nc.sync.dma_start(ka[:, n_cache:n_kv_pad], kt_d[s][:])
                ksw = work.tile([P, L], f32, tag="swstage", bufs=1,
                                name=f"ksw{s}")
                nc.sync.dma_start(ksw[:], kts_d[s][:])
                rope(ka[:, n_cache:n_cache + L], ka[:, n_cache:n_cache + L],
                     ksw[:, :], L, 0)
                # values: [n_kv_pad, d] rows -> [P, KT*d] tiles
                src = va_d[s][:].rearrange("(t p) d -> p t d", p=P)
                dst = va[:].rearrange("p (t d) -> p t d", d=d)
                qtr = KT // 8
                for cidx in range(8):
                    t0 = cidx * qtr
                    t1_ = KT if cidx == 7 else (cidx + 1) * qtr
                    nc.sync.dma_start(dst[:, t0:t1_, :], src[:, t0:t1_, :])

            load_kv_slot("a")

            # B-pass DVE softmax-denominator accumulators (ping-pong)
            sacc = [big.tile([P, QBLK], f32, tag=f"sacc{i}", name=f"sacc{i}")
                    for i in range(2)]

            def run_pass(pidx, slot, q0, dve_frac=4):
                """One 768-wide q pass. Software-pipelined one k-tile deep:
                S^T(kt+1) is issued before AV/sums(kt) so TensorE never stalls
                on exp(kt). Softmax denominators: k-tiles with kt % 5 <
                dve_frac accumulate on DVE (ping-pong adds), the rest via
                ones-matmul on TensorE; both fold into sums_ps at pass end."""
                ot_ps = psum.tile([P, QBLK], f32, tag="ot", name=f"ot{pidx}")
                sums_ps = psum.tile([P, QBLK], f32, tag="sums", name=f"sums{pidx}")

                pts = {}
                state = dict(pe_first=True, n_dve=0)

                def st_mm(kt):
                    ksl = ka[:, kt * P:(kt + 1) * P]
                    sc = psum.tile([P, QBLK], f32, tag="sc", bufs=2,
                                   name=f"sc{pidx}_{kt}")
                    for (co, cw) in CHUNKS:
                        nc.tensor.matmul(sc[:, co:co + cw], ksl,
                                         rq[:, q0 + co:q0 + co + cw],
                                         start=True, stop=True)
                    pt = work.tile([P, QBLK], f32r, tag="pt", bufs=4,
                                   name=f"pt{pidx}_{kt}")
                    nc.scalar.activation(pt[:], sc[:, :], Exp, scale=scale)
                    pts[kt] = pt

                def av_sums(kt):
                    pt = pts.pop(kt)
                    vsl = va[:, kt * d:(kt + 1) * d]
                    first, last = kt == 0, kt == KT - 1
                    for (co, cw) in CHUNKS:
                        nc.tensor.matmul(ot_ps[:, co:co + cw], vsl,
                                         pt[:, co:co + cw],
                                         start=first, stop=last)
                    on_dve = kt % 5 < dve_frac and kt != KT - 1
                    if on_dve:
                        n = state["n_dve"]
                        if n == 0:
                            nc.vector.tensor_copy(sacc[0][:], pt[:].bitcast(f32))
                        else:
                            nc.vector.tensor_add(sacc[n % 2][:],
                                                 sacc[(n + 1) % 2][:],
                                                 pt[:].bitcast(f32))
                        state["n_dve"] = n + 1
                    else:
                        onemat = (onesm[:, P:2 * P] if kt == KT - 1
                                  else onesm[:, 0:P])
                        for (co, cw) in CHUNKS:
                            nc.tensor.matmul(sums_ps[:, co:co + cw], onemat,
                                             pt[:, co:co + cw],
                                             start=state["pe_first"], stop=False)
                        state["pe_first"] = False

                st_mm(0)
                for kt in range(KT):
                    if kt + 1 < KT:
                        st_mm(kt + 1)
                    av_sums(kt)
                # fold the DVE accumulator into sums_ps
                saccr = work.tile([P, QBLK], f32r, tag="saccr", bufs=1,
                                  name=f"saccr{pidx}")
                nc.vector.tensor_copy(saccr[:],
                                      sacc[(state["n_dve"] + 1) % 2][:])
                for (co, cw) in CHUNKS:
                    nc.tensor.matmul(sums_ps[:, co:co + cw], onesm[:, 0:P],
                                     saccr[:, co:co + cw],
                                     start=False, stop=True)

                # ---- drain: transpose + normalize + store ----
                ot_sb = work.tile([P, QBLK], f32, tag="otsb", name=f"otsb{pidx}")
                nc.vector.tensor_copy(ot_sb[:], ot_ps[:, :])
                # sums rows are all identical; keep lane 0
                s_sb = work.tile([1, QBLK], f32, tag="ssb", name=f"ssb{pidx}")
                nc.vector.tensor_copy(s_sb[0:1, :], sums_ps[0:1, :])
                for j in range(QBLK // P):
                    tp = psum.tile([P, P + 1], f32, tag="sc", bufs=2,
                                   name=f"tp{pidx}_{j}")
                    nc.tensor.transpose(tp[:, 0:P],
                                        ot_sb[:, j * P:(j + 1) * P], ident[:])
                    nc.tensor.transpose(tp[:, P:P + 1],
                                        s_sb[0:1, j * P:(j + 1) * P],
                                        ident[0:1, 0:1])
                    r_sb = work.tile([P, 1], f32, tag="rsb", name=f"rsb{pidx}_{j}")
                    nc.vector.reciprocal(r_sb[:], tp[:, P:P + 1])
                    o_sb = work.tile([P, d], f32, tag="osb", bufs=3,
                                     name=f"osb{pidx}_{j}")
                    nc.vector.tensor_scalar_mul(o_sb[:], tp[:, 0:P], r_sb[:])
                    row0 = q0 + j * P
                    nc.sync.dma_start(out_d[row0:row0 + P, :], o_sb[:])

            run_pass(0, "a", 0)
            run_pass(1, "a", QBLK)
            load_kv_slot("b")
            run_pass(2, "b", ablk)

    nc.finalize()
    meta = dict(QT_N=QT_N, ablk=ablk, n_kv_pad=n_kv_pad, last_valid=last_valid)
    return nc, meta


# ----------------------------------------------------------------------------
# host wrapper
# ----------------------------------------------------------------------------

def kernel(q, k, v, k_cache, v_cache, current_start, global_end_index,
           local_end_index, grid_f, grid_h, grid_w):
    from concourse.bass_utils import run_bass_kernel_spmd

    q = np.asarray(q, dtype=np.float32)
    k = np.asarray(k, dtype=np.float32)
    v = np.asarray(v, dtype=np.float32)
    k_cache = np.asarray(k_cache, dtype=np.float32)
    v_cache = np.asarray(v_cache, dtype=np.float32)
    current_start = int(current_start)
    global_end_index = int(global_end_index)
    local_end_index = int(local_end_index)
    grid_h, grid_w = int(grid_h), int(grid_w)

    B, L, n_heads, d = q.shape
    cache_len = k_cache.shape[1]
    frame_seqlen = grid_h * grid_w
    start_frame = current_start // frame_seqlen

    segs, local_end, kv_start = _plan_cache_segments(
        current_start, global_end_index, local_end_index, L, cache_len,
        frame_seqlen)
    n_cache = sum(hi - lo for lo, hi in segs)
    n_kv = n_cache + L

    key = (L, d, n_cache, n_kv)
    if key not in _BUILD_CACHE:
        _BUILD_CACHE[key] = _build_program(L, d, n_cache, n_kv)
    nc, meta = _BUILD_CACHE[key]
    QT_N, ablk, n_kv_pad = meta["QT_N"], meta["ablk"], meta["n_kv_pad"]
    last_valid = meta["last_valid"]

    # gather the cache window once (numpy)
    kc_full = np.concatenate([k_cache[0, lo:hi] for lo, hi in segs], axis=0)
    vc_full = np.concatenate([v_cache[0, lo:hi] for lo, hi in segs], axis=0)

    cos_t, sin_t = _rope_cos_sin(L, d, grid_h, grid_w, start_frame)  # [L, 64]
    H = d // 2
    perm = np.concatenate([np.arange(0, d, 2), np.arange(1, d, 2)])


    onesm = np.zeros((P, 2 * P), dtype=np.float32)
    onesm[:, 0:P] = 1.0
    onesm[0:last_valid, P:2 * P] = 1.0
    ident = np.eye(P, dtype=np.float32)

    perm_swap = np.concatenate([np.arange(1, d, 2), np.arange(0, d, 2)])

    def dei_T(x):  # [rows, d] -> de-interleaved transpose [d, rows]
        return np.ascontiguousarray(x.T[perm])

    def dei_T_swap(x):  # half-swapped variant: [odds; evens]
        return np.ascontiguousarray(x.T[perm_swap])

    half = L // 2
    n_pairs = n_heads // 3
    assert n_heads % 3 == 0 and n_pairs * 2 == 8, "sharding expects 12 heads/8 cores"

    in_maps = []
    core_heads = []
    for c in range(8):
        p, s = c // 2, c % 2
        headA = 3 * p if s == 0 else 3 * p + 2
        headB = 3 * p + 1
        qsl = slice(0, half) if s == 0 else slice(half, L)
        core_heads.append((headA, headB, qsl))

        cosq = np.ones((P, QT_N), dtype=np.float32)
        sinq = np.zeros((P, QT_N), dtype=np.float32)
        for (c0, tab) in ((0, slice(0, L)), (ablk, qsl)):
            ct, st = cos_t[tab].T, sin_t[tab].T
            w = ct.shape[1]
            cosq[0:H, c0:c0 + w] = ct
            cosq[H:P, c0:c0 + w] = ct
            sinq[0:H, c0:c0 + w] = -st
            sinq[H:P, c0:c0 + w] = st

        qt = np.zeros((P, QT_N), dtype=np.float32)
        qt[:, 0:L] = dei_T(q[0, :, headA, :])
        qt[:, ablk:ablk + half] = dei_T(q[0, qsl, headB, :])
        qts = np.zeros((P, QT_N), dtype=np.float32)
        qts[:, 0:L] = dei_T_swap(q[0, :, headA, :])
        qts[:, ablk:ablk + half] = dei_T_swap(q[0, qsl, headB, :])

        im = {"qt": qt, "qts": qts, "cosq": cosq, "sinq": sinq,
              "onesm": onesm, "ident": ident}
        for tag, h in (("a", headA), ("b", headB)):
            ktn = np.zeros((P, n_kv_pad - n_cache), dtype=np.float32)
            ktn[:, 0:L] = dei_T(k[0, :, h, :])
            im[f"kt{tag}"] = ktn
            im[f"kts{tag}"] = dei_T_swap(k[0, :, h, :])
            im[f"kc{tag}"] = dei_T(kc_full[:, h, :])
            vaa = np.zeros((n_kv_pad, d), dtype=np.float32)
            vaa[0:n_cache] = vc_full[:, h, :]
            vaa[n_cache:n_cache + L] = v[0, :, h, :]
            im[f"va{tag}"] = vaa
        in_maps.append(im)

    res = run_bass_kernel_spmd(nc, in_maps, core_ids=list(range(8)))

    out = np.empty((B, L, n_heads, d), dtype=np.float32)
    for c in range(8):
        headA, headB, qsl = core_heads[c]
        o = res.results[c]["o"]
        out[0, :, headA, :] = o[0:L]
        out[0, qsl, headB, :] = o[ablk:ablk + half]
    return out
